# revision 33
# baseline (speedup 1.0000x reference)
"""Multi-head causal attention on 8 TRN2 NeuronCores.

Sharding: core c -> (batch b = c//2, head-group g = c%2). Each core computes
Q/K/V projections for its 8 heads (512 of the 1024 channels), causal
attention, and the row-parallel W_o partial product; the host sums the two
partials per batch (the "all-reduce").

Device layouts (per core):
  xT   (1024, 2048) bf16   x[b] transposed (channels on partitions)
  wqT  (1024, 512)  bf16   W_q[rows g].T  -> lhsT for QT = Wq_g @ xT
  wkT  (1024, 512)  bf16   same for K
  wvT  (1024, 512)  bf16   rhs for natural-layout V = x @ Wv_g.T
  woT  (512, 1024)  bf16   W_o[:, cols g].T -> lhsT for yT = Wo_g @ O^T
  mask (128, 2048)  bf16   4 diagonal-block masks (128x512 each)
  yT   (1024, 2048) f32    partial output, transposed

Attention per head h (d_k=64): scores are computed transposed,
S^T = K_h @ Q_h^T (k on partitions, q on free axis), exp on the scalar
engine (no max subtraction: |scores/8| < ~6 at these scales), multiplicative
0/1 mask on diagonal blocks only. P^T is consumed directly as the moving
operand of out^T = [V_h | 1*64] ^T @ P^T: the stationary operand is widened
to 128 columns, cols 64:128 all-ones, so PSUM rows 64:128 all accumulate the
softmax denominator Z (matmul cost is set by the moving columns, so the
widening is free).  Normalize is then reciprocal([64,512]) + mul([64,512])
on the vector engine -- no 1-partition ops, no gpsimd broadcast.
Heads run in pairs (partition offsets 0/64) so the two K=64 score matmuls
occupy disjoint PE row-groups concurrently.

Pipelining (v2): the whole kernel is one flat stream of attention steps
(a, j, k0).  The scores matmul for step i+1 is emitted BEFORE the AV
matmul of step i, so the scalar engine's exp runs back-to-back instead of
serializing behind the S->exp->mask->AV chain each step (PE MMs issue
strictly FIFO).  All projection / V-tile / W_o work that is not needed to
start step 0 is a "filler" queue pulled into the PE stream at a per-pair
rate, with per-block prerequisite markers forced before each block's first
scores matmul.  Input DMA is split across the two HWDGE queues (sync: x
chunks, scalar: weights); a few dummy matmuls at t=0 keep the PE HAM
clock-gate warm before real work lands.
"""

from collections import deque

import numpy as np

B, T, D = 4, 2048, 1024
NH, DK = 16, 64
NCORES = 8
HPC = NH // 2            # heads per core
HD = HPC * DK            # 512 head-dim channels per core
P = 128                  # partitions
NT = T // P              # 16 k-tiles
NQ = T // 512            # 4 q-blocks

_CACHE = {}


def _build():
    import concourse.mybir as mybir
    import concourse.tile as tile
    from concourse import bacc
    from concourse.tile import add_dep_helper

    f32, bf16 = mybir.dt.float32, mybir.dt.bfloat16
    Exp = mybir.ActivationFunctionType.Exp

    nc = bacc.Bacc(None, target_bir_lowering=False, debug=False)
    xT = nc.dram_tensor("xT", [D, T], bf16, kind="ExternalInput")
    wqT = nc.dram_tensor("wqT", [D, HD], bf16, kind="ExternalInput")
    wkT = nc.dram_tensor("wkT", [D, HD], bf16, kind="ExternalInput")
    wvT = nc.dram_tensor("wvT", [D, HD], bf16, kind="ExternalInput")
    woT = nc.dram_tensor("woT", [HD, D], bf16, kind="ExternalInput")
    mask = nc.dram_tensor("mask", [P, 4 * 1024], bf16, kind="ExternalInput")
    yT = nc.dram_tensor("yT", [D, T], bf16, kind="ExternalOutput")

    with tile.TileContext(nc) as tc:
        with (
            tc.tile_pool(name="persist", bufs=1) as persist,
            tc.tile_pool(name="work", bufs=6) as work,
            tc.tile_pool(name="psum", bufs=2, space="PSUM") as psum,
            tc.tile_pool(name="psum2", bufs=2, space="PSUM") as psum2,
        ):
            # ---- persistent tiles --------------------------------------
            # x chunks: separate tiles per (c, t-range) so each is written
            # by exactly one DMA at SBUF offset 0.
            XCH = ((0, 512), (512, 1024), (1024, 2048))
            xtc = [[persist.tile([P, hi - lo], bf16, tag=f"x{c}_{lo}",
                                 name=f"x{c}_{lo}")
                    for (lo, hi) in XCH]
                   for c in range(8)]

            def xsl(c, col0, width):
                # slice [col0, col0+width) of core-chunk c across chunk tiles
                for ci, (lo, hi) in enumerate(XCH):
                    if lo <= col0 and col0 + width <= hi:
                        return xtc[c][ci][:, col0 - lo:col0 - lo + width]
                raise AssertionError((col0, width))
            wq_sb = persist.tile([P, 8, HD], bf16, tag="wq")
            wk_sb = persist.tile([P, 8, HD], bf16, tag="wk")
            wv_sb = persist.tile([P, 8, HD], bf16, tag="wv")
            wo_sb = persist.tile([P, 4, D], bf16, tag="wo")
            mask_sb = persist.tile([P, 4, 2, 512], bf16, tag="mask")
            qt = [persist.tile([P, T], bf16, tag=f"qt{a}", name=f"qt{a}")
                  for a in range(4)]
            kt = [persist.tile([P, T], bf16, tag=f"kt{a}", name=f"kt{a}")
                  for a in range(4)]
            # V tile: per head 128 cols = [V_h (64) | ones (64)]; the ones
            # columns make AV's PSUM rows 64:128 accumulate Z on 64 rows.
            vt = [persist.tile([P, HPC, 2 * DK], bf16, tag=f"v{tt}",
                               name=f"v{tt}")
                  for tt in range(NT)]
            otn = [persist.tile([P, T], bf16, tag=f"otn{i}", name=f"otn{i}")
                   for i in range(4)]
            warm = persist.tile([P, 512], bf16, tag="warm")

            # ---- PE warm-up: dummy matmuls from t=0 so the HAM clock
            # gate reaches 8/8 before the first real matmul lands.
            nc.vector.memset(warm, 0.0)
            for i in range(8):
                wps = psum.tile([P, 512], f32, tag="ps", name=f"warmps{i}")
                nc.tensor.matmul(wps, lhsT=warm[:, 0:P], rhs=warm,
                                 start=True, stop=True)
            for tt in range(NT):
                nc.vector.memset(vt[tt][:, :, DK:2 * DK], 1.0)

            # ---- input DMAs --------------------------------------------
            def xdma_start(c, ci, after):
                lo, hi = XCH[ci]
                ins = nc.sync.dma_start(
                    out=xtc[c][ci],
                    in_=xT[P * c:P * c + P, lo:hi],
                )
                if after is not None:
                    add_dep_helper(ins.ins, after, sync=True,
                                   reason="x chunk order")
                return ins.ins

            # x alone on the sync queue (8 per-c chains, chunk-major so the
            # first 1MB lands fast); all weights on the scalar-engine HWDGE
            # queue in consumption order (wv -> wq -> wk -> mask -> wo).
            # The two queues stream concurrently.
            prev = {}
            for ci in range(3):
                for c in range(8):
                    prev[c] = xdma_start(c, ci, prev.get(c))
            wprev = None
            for dst, srcp in ((wv_sb, wvT), (wq_sb, wqT), (wk_sb, wkT),
                              (mask_sb, None), (wo_sb, woT)):
                if srcp is None:
                    ins = nc.scalar.dma_start(
                        out=mask_sb,
                        in_=mask.rearrange("p (r g q) -> p r g q",
                                           g=2, q=512),
                    ).ins
                else:
                    ins = nc.scalar.dma_start(
                        out=dst,
                        in_=srcp.rearrange("(co p) d -> p co d", p=P),
                    ).ins
                if wprev is not None:
                    add_dep_helper(ins, wprev, sync=True,
                                   reason="weight order")
                wprev = ins

            # ---- op builders (each closure emits one PE matmul) --------
            def v_tile_ops(tt):
                st = {}

                def mk(c):
                    def op():
                        if c == 0:
                            st["ps"] = psum.tile([P, HD], f32, tag="ps",
                                                 name=f"vps{tt}")
                        nc.tensor.matmul(
                            st["ps"],
                            lhsT=xsl(c, P * tt, P),
                            rhs=wv_sb[:, c, :],
                            start=(c == 0),
                            stop=(c == 7),
                        )
                        if c == 7:
                            nc.vector.tensor_copy(
                                vt[tt][:, :, 0:DK],
                                st["ps"].rearrange("p (h e) -> p h e", e=DK),
                            )
                    return op

                return [mk(c) for c in range(8)]

            def proj_tile_ops(nm, w_sb, out_sb, a, tch):
                st = {}

                def mk(c):
                    def op():
                        if c == 0:
                            st["ps"] = psum.tile([P, 512], f32, tag="ps",
                                                 name=f"{nm}ps{a}_{tch}")
                        nc.tensor.matmul(
                            st["ps"],
                            lhsT=w_sb[:, c, 128 * a:128 * a + 128],
                            rhs=xsl(c, 512 * tch, 512),
                            start=(c == 0),
                            stop=(c == 7),
                        )
                        if c == 7:
                            nc.vector.tensor_copy(
                                out_sb[:, 512 * tch:512 * tch + 512], st["ps"]
                            )
                    return op

                return [mk(c) for c in range(8)]

            def wo_tile_ops(dt_, tch):
                st = {}

                def mk(c):
                    def op():
                        if c == 0:
                            st["ps"] = psum.tile([P, 512], f32, tag="ps",
                                                 name=f"yps{dt_}_{tch}")
                        nc.tensor.matmul(
                            st["ps"],
                            lhsT=wo_sb[:, c, 128 * dt_:128 * dt_ + 128],
                            rhs=otn[c][:, 512 * tch:512 * tch + 512],
                            start=(c == 0),
                            stop=(c == 3),
                        )
                        if c == 3:
                            yst = work.tile([P, 512], bf16, tag="yst", bufs=3,
                                            name=f"yst{dt_}_{tch}")
                            nc.vector.tensor_copy(yst, st["ps"])
                            nc.sync.dma_start(
                                out=yT[128 * dt_:128 * dt_ + 128,
                                       512 * tch:512 * tch + 512],
                                in_=yst,
                            )
                    return op

                return [mk(c) for c in range(4)]

            # ---- upfront minimum: V tiles 0-3 + pair-0 projections for
            # t-chunk 0.  Everything else goes through the filler queue.
            for tt in range(4):
                for op in v_tile_ops(tt):
                    op()
            for op in proj_tile_ops("qt", wq_sb, qt[0], 0, 0):
                op()
            for op in proj_tile_ops("kt", wk_sb, kt[0], 0, 0):
                op()

            # ---- filler queue in dependency order, with per-block
            # prerequisite markers (filler count that must be emitted
            # before that block's first scores matmul).
            # j-major consumption order: col 0 needs pairs 1-3's t-chunk-0
            # projections; before col j: all pairs' chunk-j projections and
            # V tiles 4j..4j+3.
            fillers = deque()
            marker = {}
            vt_marker = {tt: 0 for tt in range(4)}
            for a in range(1, 4):
                fillers.extend(proj_tile_ops("qt", wq_sb, qt[a], a, 0))
                fillers.extend(proj_tile_ops("kt", wk_sb, kt[a], a, 0))
                marker[(a, 0)] = len(fillers)
            for tch in range(1, 4):
                for a in range(4):
                    fillers.extend(proj_tile_ops("qt", wq_sb, qt[a], a, tch))
                    fillers.extend(proj_tile_ops("kt", wk_sb, kt[a], a, tch))
                    marker[(a, tch)] = len(fillers)
                for tt in range(4 * tch, 4 * tch + 4):
                    fillers.extend(v_tile_ops(tt))
                    vt_marker[tt] = len(fillers)

            fillers_wo = deque()
            pulled = [0]

            def pull_main(n):
                for _ in range(n):
                    if fillers:
                        fillers.popleft()()
                        pulled[0] += 1

            def pull(n):
                # prefer dependency-ordered main fillers, then W_o tiles
                for _ in range(n):
                    if fillers:
                        fillers.popleft()()
                        pulled[0] += 1
                    elif fillers_wo:
                        fillers_wo.popleft()()

            def ensure(need):
                # PE matmuls execute strictly FIFO: anything a matmul
                # depends on MUST be emitted before it (else deadlock).
                pull_main(need - pulled[0])

            # ---- attention: flat software-pipelined step stream --------
            # j-major: W_o chunk j follows column j, spreading the output
            # projection and its DMA across the whole attention phase.
            blocks = [(a, j) for j in range(NQ) for a in range(4)]
            steps = [(a, j, k0) for (a, j) in blocks for k0 in range(4 * j + 4)]
            pull_rate = {0: 4, 1: 4, 2: 3, 3: 2}  # keyed by column j

            av_t = {}
            s_ps = {}
            u_t = {}

            def emit_s(i):
                a, j, k0 = steps[i]
                r = k0 - 4 * j
                lo = 128 * r if r > 0 else 0
                if k0 == 0:
                    ensure(marker.get((a, j), 0))
                sp = psum2.tile([P, 2, 512], f32, tag="s2",
                                name=f"sps{a}_{j}_{k0}")
                s_ps[i] = (sp, lo)
                for hh in (0, 1):
                    poff = 64 * hh
                    nc.tensor.matmul(
                        sp[:, hh, lo:512],
                        lhsT=kt[a][poff:poff + 64, P * k0:P * k0 + P],
                        rhs=qt[a][poff:poff + 64,
                                  512 * j + lo:512 * j + 512],
                        start=True,
                        stop=True,
                    )

            emit_s(0)
            for i, (a, j, k0) in enumerate(steps):
                r = k0 - 4 * j
                sp, lo = s_ps.pop(i)
                ut = work.tile([P, 2, 512], bf16, tag="u", bufs=8,
                               name=f"u{a}_{j}_{k0}")
                nc.scalar.activation(
                    ut[:, :, lo:512], sp[:, :, lo:512], Exp, scale=0.125,
                )
                if r >= 0:
                    nc.vector.tensor_mul(
                        ut[:, :, lo:512],
                        ut[:, :, lo:512],
                        mask_sb[:, r, :, lo:512],
                    )
                if i + 1 < len(steps):
                    emit_s(i + 1)
                # extra pulls at block starts keep the PE fed while the
                # new block's first AV waits out the av-buffer WAR on the
                # previous block's normalize chain.
                pull(5 if k0 == 0 else pull_rate[j])
                ensure(vt_marker.get(k0, 0))
                if k0 == 0:
                    av_t[0] = psum.tile([P, 512], f32, tag="av", bufs=2,
                                        name=f"av{a}_0_{j}")
                    av_t[1] = psum.tile([P, 512], f32, tag="av", bufs=2,
                                        name=f"av{a}_1_{j}")
                for hh in (0, 1):
                    nc.tensor.matmul(
                        av_t[hh][:, lo:512],
                        lhsT=vt[k0][:, 2 * a + hh, :],
                        rhs=ut[:, hh, lo:512],
                        start=(k0 == 0),
                        stop=(k0 == 4 * j + 3),
                    )
                if k0 == 4 * j + 3:
                    # ---- normalize: otn = av[0:64] * recip(Z rows) -----
                    # (Z staged via SBUF: the custom-DVE reciprocal reads
                    # garbage from PSUM on HW, though CoreSim accepts it.)
                    for hh in (0, 1):
                        zz = work.tile([DK, 512], f32, tag="zz", bufs=4,
                                       name=f"zz{a}_{j}_{hh}")
                        nc.vector.tensor_copy(zz, av_t[hh][DK:2 * DK, :])
                        rz = work.tile([DK, 512], f32, tag="rz", bufs=4,
                                       name=f"rz{a}_{j}_{hh}")
                        nc.vector.reciprocal_approx_fast(rz, zz)
                        nc.vector.tensor_mul(
                            otn[a][64 * hh:64 * hh + 64,
                                   512 * j:512 * j + 512],
                            av_t[hh][0:DK, :],
                            rz,
                        )
                    if a == 3:
                        for dt_ in range(8):
                            fillers_wo.extend(wo_tile_ops(dt_, j))

            # ---- tail: whatever W_o work wasn't woven in ---------------
            pull(len(fillers) + len(fillers_wo))

    nc.finalize()
    return nc


def _get_nc():
    if "nc" not in _CACHE:
        _CACHE["nc"] = _build()
    return _CACHE["nc"]


def kernel(x, W_q, W_k, W_v, W_o):
    import ml_dtypes
    from concourse.bass_utils import run_bass_kernel_spmd

    bf16 = ml_dtypes.bfloat16
    x = np.asarray(x, dtype=np.float32)
    W_q = np.asarray(W_q, dtype=np.float32)
    W_k = np.asarray(W_k, dtype=np.float32)
    W_v = np.asarray(W_v, dtype=np.float32)
    W_o = np.asarray(W_o, dtype=np.float32)

    kk = np.arange(P)[:, None]
    qq = np.arange(512)[None, :]
    mask = np.concatenate(
        [np.tile(qq >= kk + 128 * r, (1, 2)) for r in range(4)], axis=1
    ).astype(bf16)

    in_maps = []
    for c in range(NCORES):
        b, g = c // 2, c % 2
        rows = slice(HD * g, HD * g + HD)
        in_maps.append(
            {
                "xT": np.ascontiguousarray(x[b].T).astype(bf16),
                "wqT": np.ascontiguousarray(W_q[rows, :].T).astype(bf16),
                "wkT": np.ascontiguousarray(W_k[rows, :].T).astype(bf16),
                "wvT": np.ascontiguousarray(W_v[rows, :].T).astype(bf16),
                "woT": np.ascontiguousarray(W_o[:, rows].T).astype(bf16),
                "mask": mask,
            }
        )

    res = run_bass_kernel_spmd(_get_nc(), in_maps, list(range(NCORES)))
    y = np.zeros((B, T, D), np.float32)
    for c in range(NCORES):
        y[c // 2] += res.results[c]["yT"].T.astype(np.float32)
    return y


# revision 36
# speedup vs baseline: 1.0357x; 1.0357x over previous
"""Multi-head causal attention on 8 TRN2 NeuronCores.

Sharding: core c -> (batch b = c//2, head-group g = c%2). Each core computes
Q/K/V projections for its 8 heads (512 of the 1024 channels), causal
attention, and the row-parallel W_o partial product; the host sums the two
partials per batch (the "all-reduce").

Device layouts (per core):
  xT   (1024, 2048) bf16   x[b] transposed (channels on partitions)
  wqT  (1024, 512)  bf16   W_q[rows g].T  -> lhsT for QT = Wq_g @ xT
  wkT  (1024, 512)  bf16   same for K
  wvT  (1024, 512)  bf16   rhs for natural-layout V = x @ Wv_g.T
  woT  (512, 1024)  bf16   W_o[:, cols g].T -> lhsT for yT = Wo_g @ O^T
  mask (128, 2048)  bf16   4 diagonal-block masks (128x512 each)
  yT   (1024, 2048) f32    partial output, transposed

Attention per head h (d_k=64): scores are computed transposed,
S^T = K_h @ Q_h^T (k on partitions, q on free axis), exp on the scalar
engine (no max subtraction: |scores/8| < ~6 at these scales), multiplicative
0/1 mask on diagonal blocks only. P^T is consumed directly as the moving
operand of out^T = [V_h | 1*64] ^T @ P^T: the stationary operand is widened
to 128 columns, cols 64:128 all-ones, so PSUM rows 64:128 all accumulate the
softmax denominator Z (matmul cost is set by the moving columns, so the
widening is free).  Normalize is then reciprocal([64,512]) + mul([64,512])
on the vector engine -- no 1-partition ops, no gpsimd broadcast.
Heads run in pairs (partition offsets 0/64) so the two K=64 score matmuls
occupy disjoint PE row-groups concurrently.

Pipelining (v2): the whole kernel is one flat stream of attention steps
(a, j, k0).  The scores matmul for step i+1 is emitted BEFORE the AV
matmul of step i, so the scalar engine's exp runs back-to-back instead of
serializing behind the S->exp->mask->AV chain each step (PE MMs issue
strictly FIFO).  All projection / V-tile / W_o work that is not needed to
start step 0 is a "filler" queue pulled into the PE stream at a per-pair
rate, with per-block prerequisite markers forced before each block's first
scores matmul.  Input DMA is split across the two HWDGE queues (sync: x
chunks, scalar: weights); a few dummy matmuls at t=0 keep the PE HAM
clock-gate warm before real work lands.
"""

from collections import deque

import numpy as np

B, T, D = 4, 2048, 1024
NH, DK = 16, 64
NCORES = 8
HPC = NH // 2            # heads per core
HD = HPC * DK            # 512 head-dim channels per core
P = 128                  # partitions
NT = T // P              # 16 k-tiles
NQ = T // 512            # 4 q-blocks

_CACHE = {}


def _build():
    import concourse.mybir as mybir
    import concourse.tile as tile
    from concourse import bacc
    from concourse.tile import add_dep_helper

    f32, bf16 = mybir.dt.float32, mybir.dt.bfloat16
    Exp = mybir.ActivationFunctionType.Exp

    nc = bacc.Bacc(None, target_bir_lowering=False, debug=False)
    xT = nc.dram_tensor("xT", [D, T], bf16, kind="ExternalInput")
    wqT = nc.dram_tensor("wqT", [D, HD], bf16, kind="ExternalInput")
    wkT = nc.dram_tensor("wkT", [D, HD], bf16, kind="ExternalInput")
    wvT = nc.dram_tensor("wvT", [D, HD], bf16, kind="ExternalInput")
    woT = nc.dram_tensor("woT", [HD, D], bf16, kind="ExternalInput")
    mask = nc.dram_tensor("mask", [P, 4 * 1024], bf16, kind="ExternalInput")
    yT = nc.dram_tensor("yT", [D, T], bf16, kind="ExternalOutput")

    with tile.TileContext(nc) as tc:
        with (
            tc.tile_pool(name="persist", bufs=1) as persist,
            tc.tile_pool(name="work", bufs=6) as work,
            tc.tile_pool(name="psum", bufs=2, space="PSUM") as psum,
            tc.tile_pool(name="psum2", bufs=2, space="PSUM") as psum2,
        ):
            # ---- persistent tiles --------------------------------------
            # x chunks: separate tiles per (c, t-range) so each is written
            # by exactly one DMA at SBUF offset 0.
            XCH = ((0, 512), (512, 1024), (1024, 2048))
            xtc = [[persist.tile([P, hi - lo], bf16, tag=f"x{c}_{lo}",
                                 name=f"x{c}_{lo}")
                    for (lo, hi) in XCH]
                   for c in range(8)]

            def xsl(c, col0, width):
                # slice [col0, col0+width) of core-chunk c across chunk tiles
                for ci, (lo, hi) in enumerate(XCH):
                    if lo <= col0 and col0 + width <= hi:
                        return xtc[c][ci][:, col0 - lo:col0 - lo + width]
                raise AssertionError((col0, width))
            wq_sb = persist.tile([P, 8, HD], bf16, tag="wq")
            wk_sb = persist.tile([P, 8, HD], bf16, tag="wk")
            wv_sb = persist.tile([P, 8, HD], bf16, tag="wv")
            wo_sb = persist.tile([P, 4, D], bf16, tag="wo")
            mask_sb = persist.tile([P, 4, 2, 512], bf16, tag="mask")
            qt = [persist.tile([P, T], bf16, tag=f"qt{a}", name=f"qt{a}")
                  for a in range(4)]
            kt = [persist.tile([P, T], bf16, tag=f"kt{a}", name=f"kt{a}")
                  for a in range(4)]
            # V tile: per head 128 cols = [V_h (64) | ones (64)]; the ones
            # columns make AV's PSUM rows 64:128 accumulate Z on 64 rows.
            vt = [persist.tile([P, HPC, 2 * DK], bf16, tag=f"v{tt}",
                               name=f"v{tt}")
                  for tt in range(NT)]
            otn = [persist.tile([P, T], bf16, tag=f"otn{i}", name=f"otn{i}")
                   for i in range(4)]
            warm = persist.tile([P, 512], bf16, tag="warm")

            # ---- PE warm-up: dummy matmuls from t=0 so the HAM clock
            # gate reaches 8/8 before the first real matmul lands.
            nc.vector.memset(warm, 0.0)
            for i in range(8):
                wps = psum.tile([P, 512], f32, tag="ps", name=f"warmps{i}")
                nc.tensor.matmul(wps, lhsT=warm[:, 0:P], rhs=warm,
                                 start=True, stop=True)
            for tt in range(NT):
                nc.vector.memset(vt[tt][:, :, DK:2 * DK], 1.0)

            # ---- input DMAs --------------------------------------------
            def xdma_start(c, ci, after):
                lo, hi = XCH[ci]
                ins = nc.sync.dma_start(
                    out=xtc[c][ci],
                    in_=xT[P * c:P * c + P, lo:hi],
                )
                if after is not None:
                    add_dep_helper(ins.ins, after, sync=True,
                                   reason="x chunk order")
                return ins.ins

            # x alone on the sync queue (8 per-c chains, chunk-major so the
            # first 1MB lands fast); all weights on the scalar-engine HWDGE
            # queue in consumption order (wv -> wq -> wk -> mask -> wo).
            # The two queues stream concurrently.
            prev = {}
            for ci in range(3):
                for c in range(8):
                    prev[c] = xdma_start(c, ci, prev.get(c))
            wprev = None
            for dst, srcp in ((wv_sb, wvT), (wq_sb, wqT), (wk_sb, wkT),
                              (mask_sb, None), (wo_sb, woT)):
                if srcp is None:
                    ins = nc.scalar.dma_start(
                        out=mask_sb,
                        in_=mask.rearrange("p (r g q) -> p r g q",
                                           g=2, q=512),
                    ).ins
                else:
                    ins = nc.scalar.dma_start(
                        out=dst,
                        in_=srcp.rearrange("(co p) d -> p co d", p=P),
                    ).ins
                if wprev is not None:
                    # ordering-only edge: a sync dep would stall the ACT
                    # engine FIFO (and exp) until the transfer completes
                    add_dep_helper(ins, wprev, sync=False,
                                   reason="weight order")
                wprev = ins

            # ---- op builders (each closure emits one PE matmul) --------
            def v_tile_ops(tt):
                st = {}

                def mk(c):
                    def op():
                        if c == 0:
                            st["ps"] = psum.tile([P, HD], f32, tag="ps",
                                                 name=f"vps{tt}")
                        nc.tensor.matmul(
                            st["ps"],
                            lhsT=xsl(c, P * tt, P),
                            rhs=wv_sb[:, c, :],
                            start=(c == 0),
                            stop=(c == 7),
                        )
                        if c == 7:
                            nc.vector.tensor_copy(
                                vt[tt][:, :, 0:DK],
                                st["ps"].rearrange("p (h e) -> p h e", e=DK),
                            )
                    return op

                return [mk(c) for c in range(8)]

            def proj_tile_ops(nm, w_sb, out_sb, a, tch):
                st = {}

                def mk(c):
                    def op():
                        if c == 0:
                            st["ps"] = psum.tile([P, 512], f32, tag="ps",
                                                 name=f"{nm}ps{a}_{tch}")
                        nc.tensor.matmul(
                            st["ps"],
                            lhsT=w_sb[:, c, 128 * a:128 * a + 128],
                            rhs=xsl(c, 512 * tch, 512),
                            start=(c == 0),
                            stop=(c == 7),
                        )
                        if c == 7:
                            nc.vector.tensor_copy(
                                out_sb[:, 512 * tch:512 * tch + 512], st["ps"]
                            )
                    return op

                return [mk(c) for c in range(8)]

            def wo_tile_ops(dt_, tch):
                st = {}

                def mk(c):
                    def op():
                        if c == 0:
                            st["ps"] = psum.tile([P, 512], f32, tag="ps",
                                                 name=f"yps{dt_}_{tch}")
                        nc.tensor.matmul(
                            st["ps"],
                            lhsT=wo_sb[:, c, 128 * dt_:128 * dt_ + 128],
                            rhs=otn[c][:, 512 * tch:512 * tch + 512],
                            start=(c == 0),
                            stop=(c == 3),
                        )
                        if c == 3:
                            yst = work.tile([P, 512], bf16, tag="yst", bufs=3,
                                            name=f"yst{dt_}_{tch}")
                            nc.vector.tensor_copy(yst, st["ps"])
                            nc.sync.dma_start(
                                out=yT[128 * dt_:128 * dt_ + 128,
                                       512 * tch:512 * tch + 512],
                                in_=yst,
                            )
                    return op

                return [mk(c) for c in range(4)]

            # last output chunk, two stages: partial over otn[0..2] staged
            # to SBUF while the final attention block runs, then one matmul
            # + add once otn[3] lands -- shrinks the serial tail.
            pst = [persist.tile([P, 512], f32, tag=f"pst{dt_}",
                                name=f"pst{dt_}")
                   for dt_ in range(8)]

            def wo_partial_ops(dt_, tch):
                st = {}

                def mk(c):
                    def op():
                        if c == 0:
                            st["ps"] = psum.tile([P, 512], f32, tag="ps",
                                                 name=f"pps{dt_}_{tch}")
                        nc.tensor.matmul(
                            st["ps"],
                            lhsT=wo_sb[:, c, 128 * dt_:128 * dt_ + 128],
                            rhs=otn[c][:, 512 * tch:512 * tch + 512],
                            start=(c == 0),
                            stop=(c == 2),
                        )
                        if c == 2:
                            nc.vector.tensor_copy(pst[dt_], st["ps"])
                    return op

                return [mk(c) for c in range(3)]

            def wo_final_ops(dt_, tch):
                def op():
                    fps = psum.tile([P, 512], f32, tag="ps",
                                    name=f"fps{dt_}_{tch}")
                    nc.tensor.matmul(
                        fps,
                        lhsT=wo_sb[:, 3, 128 * dt_:128 * dt_ + 128],
                        rhs=otn[3][:, 512 * tch:512 * tch + 512],
                        start=True,
                        stop=True,
                    )
                    yst = work.tile([P, 512], bf16, tag="yst", bufs=3,
                                    name=f"yst{dt_}_{tch}")
                    nc.vector.tensor_add(yst, pst[dt_], fps)
                    nc.sync.dma_start(
                        out=yT[128 * dt_:128 * dt_ + 128,
                               512 * tch:512 * tch + 512],
                        in_=yst,
                    )
                return [op]

            # ---- upfront minimum: V tiles 0-3 + pair-0 projections for
            # t-chunk 0.  Everything else goes through the filler queue.
            for tt in range(4):
                for op in v_tile_ops(tt):
                    op()
            for op in proj_tile_ops("qt", wq_sb, qt[0], 0, 0):
                op()
            for op in proj_tile_ops("kt", wk_sb, kt[0], 0, 0):
                op()

            # ---- filler queue in dependency order, with per-block
            # prerequisite markers (filler count that must be emitted
            # before that block's first scores matmul).
            # j-major consumption order: col 0 needs pairs 1-3's t-chunk-0
            # projections; before col j: all pairs' chunk-j projections and
            # V tiles 4j..4j+3.
            fillers = deque()
            marker = {}
            vt_marker = {tt: 0 for tt in range(4)}
            for a in range(1, 4):
                fillers.extend(proj_tile_ops("qt", wq_sb, qt[a], a, 0))
                fillers.extend(proj_tile_ops("kt", wk_sb, kt[a], a, 0))
                marker[(a, 0)] = len(fillers)
            for tch in range(1, 4):
                for a in range(4):
                    fillers.extend(proj_tile_ops("qt", wq_sb, qt[a], a, tch))
                    fillers.extend(proj_tile_ops("kt", wk_sb, kt[a], a, tch))
                    marker[(a, tch)] = len(fillers)
                for tt in range(4 * tch, 4 * tch + 4):
                    fillers.extend(v_tile_ops(tt))
                    vt_marker[tt] = len(fillers)

            fillers_wo = deque()
            pulled = [0]

            def pull_main(n):
                for _ in range(n):
                    if fillers:
                        fillers.popleft()()
                        pulled[0] += 1

            def pull(n):
                # prefer dependency-ordered main fillers, then W_o tiles
                for _ in range(n):
                    if fillers:
                        fillers.popleft()()
                        pulled[0] += 1
                    elif fillers_wo:
                        fillers_wo.popleft()()

            def ensure(need):
                # PE matmuls execute strictly FIFO: anything a matmul
                # depends on MUST be emitted before it (else deadlock).
                pull_main(need - pulled[0])

            # ---- attention: flat software-pipelined step stream --------
            # j-major: W_o chunk j follows column j, spreading the output
            # projection and its DMA across the whole attention phase.
            blocks = [(a, j) for j in range(NQ) for a in range(4)]
            steps = [(a, j, k0) for (a, j) in blocks for k0 in range(4 * j + 4)]
            pull_rate = {0: 4, 1: 4, 2: 3, 3: 2}  # keyed by column j

            av_t = {}
            s_ps = {}
            u_t = {}

            def emit_s(i):
                a, j, k0 = steps[i]
                r = k0 - 4 * j
                lo = 128 * r if r > 0 else 0
                if k0 == 0:
                    ensure(marker.get((a, j), 0))
                sp = psum2.tile([P, 2, 512], f32, tag="s2",
                                name=f"sps{a}_{j}_{k0}")
                s_ps[i] = (sp, lo)
                for hh in (0, 1):
                    poff = 64 * hh
                    nc.tensor.matmul(
                        sp[:, hh, lo:512],
                        lhsT=kt[a][poff:poff + 64, P * k0:P * k0 + P],
                        rhs=qt[a][poff:poff + 64,
                                  512 * j + lo:512 * j + 512],
                        start=True,
                        stop=True,
                    )

            emit_s(0)
            for i, (a, j, k0) in enumerate(steps):
                r = k0 - 4 * j
                sp, lo = s_ps.pop(i)
                ut = work.tile([P, 2, 512], bf16, tag="u", bufs=8,
                               name=f"u{a}_{j}_{k0}")
                nc.scalar.activation(
                    ut[:, :, lo:512], sp[:, :, lo:512], Exp, scale=0.125,
                )
                if r >= 0:
                    nc.vector.tensor_mul(
                        ut[:, :, lo:512],
                        ut[:, :, lo:512],
                        mask_sb[:, r, :, lo:512],
                    )
                if i + 1 < len(steps):
                    emit_s(i + 1)
                # extra pulls at block starts keep the PE fed while the
                # new block's first AV waits out the av-buffer WAR on the
                # previous block's normalize chain.
                pull(5 if k0 == 0 else pull_rate[j])
                ensure(vt_marker.get(k0, 0))
                if k0 == 0:
                    av_t[0] = psum.tile([P, 512], f32, tag="av", bufs=2,
                                        name=f"av{a}_0_{j}")
                    av_t[1] = psum.tile([P, 512], f32, tag="av", bufs=2,
                                        name=f"av{a}_1_{j}")
                for hh in (0, 1):
                    nc.tensor.matmul(
                        av_t[hh][:, lo:512],
                        lhsT=vt[k0][:, 2 * a + hh, :],
                        rhs=ut[:, hh, lo:512],
                        start=(k0 == 0),
                        stop=(k0 == 4 * j + 3),
                    )
                if k0 == 4 * j + 3:
                    # ---- normalize: otn = av[0:64] * recip(Z rows) -----
                    # (Z staged via SBUF: the custom-DVE reciprocal reads
                    # garbage from PSUM on HW, though CoreSim accepts it.)
                    for hh in (0, 1):
                        zz = work.tile([DK, 512], f32, tag="zz", bufs=4,
                                       name=f"zz{a}_{j}_{hh}")
                        nc.vector.tensor_copy(zz, av_t[hh][DK:2 * DK, :])
                        rz = work.tile([DK, 512], f32, tag="rz", bufs=4,
                                       name=f"rz{a}_{j}_{hh}")
                        nc.vector.reciprocal_approx_fast(rz, zz)
                        nc.vector.tensor_mul(
                            otn[a][64 * hh:64 * hh + 64,
                                   512 * j:512 * j + 512],
                            av_t[hh][0:DK, :],
                            rz,
                        )
                    if a == 2 and j == 3:
                        for dt_ in range(8):
                            fillers_wo.extend(wo_partial_ops(dt_, 3))
                    if a == 3:
                        if j == 3:
                            for dt_ in range(8):
                                fillers_wo.extend(wo_final_ops(dt_, 3))
                        else:
                            for dt_ in range(8):
                                fillers_wo.extend(wo_tile_ops(dt_, j))

            # ---- tail: whatever W_o work wasn't woven in ---------------
            pull(len(fillers) + len(fillers_wo))

    nc.finalize()
    return nc


def _get_nc():
    if "nc" not in _CACHE:
        _CACHE["nc"] = _build()
    return _CACHE["nc"]


def kernel(x, W_q, W_k, W_v, W_o):
    import ml_dtypes
    from concourse.bass_utils import run_bass_kernel_spmd

    bf16 = ml_dtypes.bfloat16
    x = np.asarray(x, dtype=np.float32)
    W_q = np.asarray(W_q, dtype=np.float32)
    W_k = np.asarray(W_k, dtype=np.float32)
    W_v = np.asarray(W_v, dtype=np.float32)
    W_o = np.asarray(W_o, dtype=np.float32)

    kk = np.arange(P)[:, None]
    qq = np.arange(512)[None, :]
    mask = np.concatenate(
        [np.tile(qq >= kk + 128 * r, (1, 2)) for r in range(4)], axis=1
    ).astype(bf16)

    in_maps = []
    for c in range(NCORES):
        b, g = c // 2, c % 2
        rows = slice(HD * g, HD * g + HD)
        in_maps.append(
            {
                "xT": np.ascontiguousarray(x[b].T).astype(bf16),
                "wqT": np.ascontiguousarray(W_q[rows, :].T).astype(bf16),
                "wkT": np.ascontiguousarray(W_k[rows, :].T).astype(bf16),
                "wvT": np.ascontiguousarray(W_v[rows, :].T).astype(bf16),
                "woT": np.ascontiguousarray(W_o[:, rows].T).astype(bf16),
                "mask": mask,
            }
        )

    res = run_bass_kernel_spmd(_get_nc(), in_maps, list(range(NCORES)))
    y = np.zeros((B, T, D), np.float32)
    for c in range(NCORES):
        y[c // 2] += res.results[c]["yT"].T.astype(np.float32)
    return y
